# revision 52
# baseline (speedup 1.0000x reference)
"""Causal self-attention with RoPE (B=2, T=1024, C=2048, H=16) on 8 TRN2
NeuronCores, head-parallel tensor sharding (2 heads per core) with
COLLECTIVE-FREE partial-sum output.

Design:
  - x^T replicated as bf16 ExternalInput; tiles DMA'd straight to SBUF.
  - RoPE cos/sin tables ride as inline (Const) tensors baked into the NEFF.
  - QKV projections tensor-parallel (weights column-sharded, bf16); RoPE via
    PSUM-copy-first + SBUF partition-shift DMA + DVE mul/add.
  - Causal attention in [tk, tq] layout: exp on ScalarE straight out of
    PSUM, softmax denominator via an all-ones-lhsT matmul, unnormalized y
    accumulated in PSUM, one reciprocal + multiply. Fully-masked k-tiles
    skipped (c0 trimming).
  - NO collectives: per 512-token block, both local heads' attention runs
    back-to-back, then the output projection contracts ONLY this core's 256
    y-channels against its [256, 2048] slice of Wo, accumulating the two
    heads in PSUM. Each core emits a FULL-shape [2048, 2048] bf16 PARTIAL
    output; the host sums the 8 partials (f32) and reshapes. This replaces
    two AllToAlls (~50-60us each on HW) + full-Wo loads with zero on-device
    communication.
Host reassembles: sum core partials (f32), reshape to [B, T, C].
"""
import numpy as np

import concourse.bass as bass
import concourse.mybir as mybir
import concourse.tile as tile
from concourse import bacc
from concourse.bass_utils import run_bass_kernel_spmd

F32 = mybir.dt.float32
F32R = mybir.dt.float32r
BF16 = mybir.dt.bfloat16

B, T, C = 2, 1024, 2048
H = 16
D = C // H            # 128
BT = B * T            # 2048
NCORES = 8
HL = H // NCORES      # heads per core = 2
CL = HL * D           # local channels = 256
ATT_SCALE = 1.0 / float(np.sqrt(D))
ROPE_BASE = 10000.0
NEG = -1.0e30

CT = C // 128         # 16 contraction tiles
TB = BT // 512        # 4 token blocks of 512


def _rope_tables():
    inv_freq = 1.0 / (ROPE_BASE ** (np.arange(0, D, 2, dtype=np.float64) / D))
    t = np.arange(T, dtype=np.float64)
    freqs = np.outer(t, inv_freq)                        # [T, D/2]
    emb = np.concatenate([freqs, freqs], axis=-1)        # [T, D]
    cos = np.cos(emb).astype(np.float32)                 # [T, D]
    sin = np.sin(emb).astype(np.float32)
    cosT = np.ascontiguousarray(cos.T)                   # [D, T]
    sinT = np.ascontiguousarray(sin.T)
    # s[p] = q[p]*ssin[p]; rope = q*cos + shift64(s) needs ssin negated on
    # the SECOND half (s2[p<64] = s[p+64] must equal -q[p+64]*sin[p])
    sgn_sinT = sinT.copy()
    sgn_sinT[D // 2:] *= -1.0
    return cosT, sgn_sinT


def _build(variant="full"):
    # variant "smallout": 1MB output (timing experiment only, wrong results)
    smallout = variant == "smallout"
    nc = bacc.Bacc("TRN2", target_bir_lowering=False, debug=False,
                   num_devices=NCORES)

    # full x^T, replicated on every core
    xt_d = nc.dram_tensor("xt", [C, BT], BF16, kind="ExternalInput").ap()
    # qkv weights column-sharded, laid out [128, CT*CL]: partition p holds
    # WT[ct*128+p, o] at free offset ct*CL+o
    wqT_d = nc.dram_tensor("wqT", [128, CT * CL], BF16, kind="ExternalInput").ap()
    wkT_d = nc.dram_tensor("wkT", [128, CT * CL], BF16, kind="ExternalInput").ap()
    wvT_d = nc.dram_tensor("wvT", [128, CT * CL], BF16, kind="ExternalInput").ap()
    # this core's 256-row slice of Wo^T: col block h*C+o = WoT[my_c0+h*128+p, o]
    woT_d = nc.dram_tensor("woT", [128, HL * C], BF16, kind="ExternalInput").ap()
    # full-shape PARTIAL output (this core's 2 heads' contribution)
    out_d = nc.dram_tensor("out", [256 if smallout else BT, C], BF16,
                           kind="ExternalOutput").ap()

    # RoPE tables baked into the NEFF (loaded to HBM at model-load time)
    import ml_dtypes
    cosT, sgn_sinT = _rope_tables()
    cos_d = nc.inline_tensor(cosT.astype(ml_dtypes.bfloat16),
                             name="ropecos")             # [128, T] bf16
    sin_d = nc.inline_tensor(sgn_sinT.astype(ml_dtypes.bfloat16),
                             name="ropesin")             # [128, T] bf16

    with tile.TileContext(nc) as tc:
        with (
            tc.tile_pool(name="wpool", bufs=1) as wpool,
            tc.tile_pool(name="const", bufs=1) as cpool,
            tc.tile_pool(name="qkv", bufs=1) as qkvpool,
            tc.tile_pool(name="xs", bufs=6) as xspool,
            tc.tile_pool(name="rope", bufs=1) as ropepool,
            tc.tile_pool(name="att", bufs=3) as attpool,
            tc.tile_pool(name="ob", bufs=6) as obpool,
        ):
            # ---- startup loads, interleaved per queue so the first weight
            # chunks + first x tile land ~2us in and PE starts immediately;
            # cos/sin/wo (needed later) ride behind the first token block ----
            wq_sb = wpool.tile([128, CT * CL], BF16, tag="wq")
            wk_sb = wpool.tile([128, CT * CL], BF16, tag="wk")
            wv_sb = wpool.tile([128, CT * CL], BF16, tag="wv")
            qs = (nc.sync, nc.scalar, nc.gpsimd)
            wsrc = ((wq_sb, wqT_d), (wk_sb, wkT_d), (wv_sb, wvT_d))
            WCH = 4                       # cts per weight chunk
            cw = WCH * CL
            # x loaded as [128, 1024] tiles (per batch), ALL issued upfront:
            # big transfers amortize the ~1us per-DMA queue overhead, and
            # full SBUF residency (64KB/partition) kills every later x wait.
            # Weight chunks are graduated (1,3,4,8 cts) so the very first
            # matmul's deps are tiny and early x tiles interleave tightly.
            xs0_tiles = []
            xi = 0
            for c in range(CT // WCH):
                for qi, (w_sb, w_d) in enumerate(wsrc):
                    qs[qi].dma_start(out=w_sb[:, c * cw:(c + 1) * cw],
                                     in_=w_d[:, c * cw:(c + 1) * cw])
                for _ in range(3):
                    xs = xspool.tile([128, 512], BF16, tag="xs", bufs=24,
                                     name=f"xs0_{xi}")
                    qs[xi % 3].dma_start(
                        out=xs[:], in_=xt_d[xi * 128:(xi + 1) * 128, 0:512])
                    xs0_tiles.append(xs)
                    xi += 1
            while xi < CT:
                xs = xspool.tile([128, 512], BF16, tag="xs", bufs=24,
                                 name=f"xs0_{xi}")
                qs[xi % 3].dma_start(
                    out=xs[:], in_=xt_d[xi * 128:(xi + 1) * 128, 0:512])
                xs0_tiles.append(xs)
                xi += 1

            cos_sb = cpool.tile([D, T], BF16, tag="cos")
            sin_sb = cpool.tile([D, T], BF16, tag="sin")
            nc.scalar.dma_start(out=cos_sb[:], in_=cos_d.ap())
            nc.sync.dma_start(out=sin_sb[:], in_=sin_d.ap())

            ones_f = cpool.tile([128, 128], F32, tag="onesf")
            nc.gpsimd.memset(ones_f[:], 1.0)
            ones_sb = cpool.tile([128, 128], BF16, tag="ones")
            nc.vector.tensor_copy(ones_sb[:], ones_f[:])
            # warm the Exp table so LoadActFuncSet isn't on the QKV->att
            # critical path
            warm = cpool.tile([128, 128], F32, tag="warm")
            nc.scalar.activation(warm[:], ones_f[:],
                                 mybir.ActivationFunctionType.Exp)

            # additive causal mask for diagonal 128x128 blocks:
            # rows=tk, cols=tq; keep (0.0) where tk <= tq else NEG
            mask_sb = cpool.tile([128, 128], F32, tag="mask")
            nc.gpsimd.memset(mask_sb[:], 0.0)
            nc.gpsimd.affine_select(
                out=mask_sb[:], in_=mask_sb[:],
                compare_op=mybir.AluOpType.is_ge,
                fill=NEG, base=0,
                pattern=[[1, 128]], channel_multiplier=-1,
            )

            # my 256-row Wo^T slice (bf16, 8KB/partition)
            wo_sb = wpool.tile([128, HL * C], BF16, tag="wo")
            nc.gpsimd.dma_start(out=wo_sb[:], in_=woT_d)

            # persistent qkv activations, split per batch so attention on
            # batch 0 doesn't false-depend on batch-1 rope writes
            qT = [[qkvpool.tile([D, T], BF16, tag=f"qT{h}{b}", name=f"qT{h}{b}")
                   for b in range(B)] for h in range(HL)]
            kT = [[qkvpool.tile([D, T], BF16, tag=f"kT{h}{b}", name=f"kT{h}{b}")
                   for b in range(B)] for h in range(HL)]
            v_sb = [qkvpool.tile([128, (T // 128) * CL], BF16, tag=f"v{b}",
                                 name=f"v{b}")
                    for b in range(B)]

            # ---- phase 1: QKV projections + rope ----
            with tc.tile_pool(name="psqkv", bufs=1, space="PSUM") as psq:
                # dummy matmuls on ones (no DMA deps) ramp the PE p-state
                # to full clock while the first weight/x DMAs land; they
                # borrow the pv0 bank (bufs=1: same bank as the real pv0)
                warm_ps = psq.tile([128, 128], F32, tag="pv0", name="warm")
                for _ in range(24):
                    nc.tensor.matmul(warm_ps[:], ones_sb[:], ones_sb[:],
                                     start=True, stop=True)
                for tb in range(TB):
                    tcol = tb * 512
                    rcol = tcol % T          # rope table column (per batch)
                    ps_q = [psq.tile([128, 512], F32, tag=f"pq{h}", name=f"pq{h}")
                            for h in range(HL)]
                    ps_k = [psq.tile([128, 512], F32, tag=f"pk{h}", name=f"pk{h}")
                            for h in range(HL)]
                    ps_v = [psq.tile([128, CL], F32, tag=f"pv{i}", name=f"pv{i}")
                            for i in range(4)]
                    for ct in range(CT):
                        if tb == 0:
                            xs = xs0_tiles[ct]
                        else:
                            xs = xspool.tile([128, 512], BF16, tag="xs",
                                             bufs=24)
                            eng = (nc.sync, nc.scalar, nc.gpsimd)[ct % 3]
                            eng.dma_start(
                                out=xs[:],
                                in_=xt_d[ct * 128:(ct + 1) * 128,
                                         tcol:tcol + 512],
                            )
                        st, sp = ct == 0, ct == CT - 1
                        for h in range(HL):
                            nc.tensor.matmul(
                                ps_q[h][:],
                                wq_sb[:, ct * CL + h * D: ct * CL + (h + 1) * D],
                                xs[:], start=st, stop=sp)
                            nc.tensor.matmul(
                                ps_k[h][:],
                                wk_sb[:, ct * CL + h * D: ct * CL + (h + 1) * D],
                                xs[:], start=st, stop=sp)
                        for i in range(4):
                            nc.tensor.matmul(
                                ps_v[i][:],
                                xs[:, i * 128:(i + 1) * 128],
                                wv_sb[:, ct * CL:(ct + 1) * CL],
                                start=st, stop=sp)
                    # Drain ALL psum banks first: 4 rope staging copies (h=0
                    # on DVE, h=1 on ActE — GpSimd cannot read PSUM), then v
                    # copies on ActE. Rope math afterwards reads the copies:
                    # s = tmp*ssin, shift64 via two HWDGE DMAs, dst =
                    # tmp*cos + shift64(s). h=0 math on DVE, h=1 on Pool.
                    bb = tb // 2
                    staged = []
                    for h in range(HL):
                        for ps, dst in ((ps_q[h], qT[h][bb]),
                                        (ps_k[h], kT[h][bb])):
                            tmp = ropepool.tile([128, 512], BF16, tag="rtmp",
                                                bufs=4)
                            if h == 0:
                                nc.vector.tensor_copy(tmp[:], ps[:])
                            else:
                                nc.scalar.activation(
                                    tmp[:], ps[:],
                                    mybir.ActivationFunctionType.Copy)
                            staged.append((tmp, dst))
                    for i in range(4):
                        gt = (tb % 2) * 4 + i
                        nc.scalar.activation(
                            v_sb[bb][:, gt * CL:(gt + 1) * CL], ps_v[i][:],
                            mybir.ActivationFunctionType.Copy)
                    # rope math in bf16 at 2x DVE rate
                    for tmp, dst in staged:
                        s = ropepool.tile([128, 512], BF16, tag="rs",
                                          bufs=4)
                        nc.vector.tensor_mul(
                            s[:], tmp[:], sin_sb[:, rcol:rcol + 512])
                        rot = ropepool.tile([128, 512], BF16, tag="rrot",
                                            bufs=4)
                        nc.sync.dma_start(out=rot[0:64, :],
                                          in_=s[64:128, :])
                        nc.scalar.dma_start(out=rot[64:128, :],
                                            in_=s[0:64, :])
                        u = ropepool.tile([128, 512], BF16, tag="ru",
                                          bufs=4)
                        nc.vector.tensor_mul(
                            u[:], tmp[:], cos_sb[:, rcol:rcol + 512])
                        nc.vector.tensor_add(
                            dst[:, rcol:rcol + 512], u[:], rot[:])

            # ---- phase 2: attention (block-outer, head-inner) + fused
            # partial output projection, pipelined one block behind so proj
            # never stalls on the reciprocal chain. The j-loop issues scores
            # two tiles ahead of the l/y matmuls so the in-order PE queue
            # rides out the exp-chain latency. ----
            def emit_proj(blk):
                b, jj, yb = blk
                row0 = 0 if smallout else b * T + jj * 512
                for tc_ in range(4):
                    for oq in range(4):
                        ps_o = psa.tile([128, 512], F32, tag="o", bufs=2)
                        for h in range(HL):
                            nc.tensor.matmul(
                                ps_o[:],
                                yb[h][:, tc_ * 128:(tc_ + 1) * 128],
                                wo_sb[:, h * C + oq * 512:
                                      h * C + (oq + 1) * 512],
                                start=(h == 0), stop=(h == HL - 1))
                        # ob copies on DVE and out-DMAs on sync/gpsimd so
                        # the Activation queue stays pure exp (any proj work
                        # queued between exps stalls the attention chain)
                        ob = obpool.tile([128, 512], BF16, tag="ob")
                        nc.vector.tensor_copy(ob[:], ps_o[:])
                        r0 = row0 + (tc_ % 2 if smallout else tc_) * 128
                        eng = (nc.sync, nc.gpsimd)[oq % 2]
                        eng.dma_start(
                            out=out_d[r0:r0 + 128,
                                      oq * 512:(oq + 1) * 512],
                            in_=ob[:])

            with tc.tile_pool(name="psatt", bufs=1, space="PSUM") as psa:
                # keep the PE clock hot across the QKV->attention drain
                warm2 = psa.tile([128, 512], F32, tag="o", bufs=2,
                                 name="warm2")
                for _ in range(14):
                    nc.tensor.matmul(warm2[:, 0:128], ones_sb[:],
                                     ones_sb[:], start=True, stop=True)
                prev_blk = None
                for b, jj in ((0, 0), (1, 0), (0, 1), (1, 1)):
                    lcol = jj * 512
                    njt = 4 * jj + 4
                    yb = [None, None]
                    for h in range(HL):
                        ps_y = psa.tile([128, 512], F32, tag="y", bufs=2)
                        ps_l = psa.tile([128, 512], F32, tag="l", bufs=1)

                        def flush(ent):
                            j, p, c0 = ent
                            st, sp = j == 0, j == njt - 1
                            nc.tensor.matmul(
                                ps_l[:, c0:512], ones_sb[:],
                                p[:, c0:512], start=st, stop=sp)
                            nc.tensor.matmul(
                                ps_y[:, c0:512],
                                v_sb[b][:, j * CL + h * D:
                                         j * CL + (h + 1) * D],
                                p[:, c0:512], start=st, stop=sp)

                        pend = []
                        for j in range(njt):
                            c0 = max(0, j * 128 - jj * 512)
                            ps_s = psa.tile([128, 512], F32, tag="s",
                                            bufs=3)
                            nc.tensor.matmul(
                                ps_s[:, c0:512],
                                kT[h][b][:, j * 128:(j + 1) * 128],
                                qT[h][b][:, lcol + c0: lcol + 512],
                                start=True, stop=True)
                            diag0 = j * 128 - jj * 512
                            if 0 <= diag0 < 512:
                                nc.vector.tensor_add(
                                    ps_s[:, diag0:diag0 + 128],
                                    ps_s[:, diag0:diag0 + 128],
                                    mask_sb[:])
                            p = attpool.tile([128, 512], BF16, tag="p",
                                             bufs=4)
                            nc.scalar.activation(
                                p[:, c0:512], ps_s[:, c0:512],
                                mybir.ActivationFunctionType.Exp,
                                scale=ATT_SCALE)
                            pend.append((j, p, c0))
                            if len(pend) > 2:
                                flush(pend.pop(0))
                            if j == 1 and h == 0 and prev_blk is not None:
                                emit_proj(prev_blk)
                                prev_blk = None
                        for ent in pend:
                            flush(ent)
                        rec = attpool.tile([128, 512], F32, tag="rec")
                        nc.vector.reciprocal(rec[:], ps_l[:])
                        # normalized y in bf16: [128 y-chans of h, 512 tok]
                        yb[h] = attpool.tile([128, 512], BF16, tag=f"yb{h}",
                                             bufs=2, name=f"yb{h}")
                        nc.vector.tensor_mul(yb[h][:], ps_y[:], rec[:])
                    prev_blk = (b, jj, yb)
                emit_proj(prev_blk)

    nc.compile()
    return nc


_NC_CACHE = None


def _get_nc():
    global _NC_CACHE
    if _NC_CACHE is None:
        _NC_CACHE = _build()
    return _NC_CACHE


def make_in_maps(x, Wq, Wk, Wv, Wo):
    import ml_dtypes

    def conv(a):
        return np.ascontiguousarray(a).astype(ml_dtypes.bfloat16)

    x = np.asarray(x, dtype=np.float32)
    xT = conv(x.reshape(BT, C).T)                        # [C, BT] bf16

    def wlay(wT, cols):
        # [C, cols] -> [128, CT*cols]: partition p holds WT[ct*128+p, :]
        return np.ascontiguousarray(
            wT.reshape(CT, 128, cols).transpose(1, 0, 2).reshape(
                128, CT * cols))

    WoT = np.asarray(Wo, dtype=np.float32).T             # [C, C]
    in_maps = []
    for m in range(NCORES):
        sl = slice(m * CL, (m + 1) * CL)
        # my Wo^T rows [m*CL, m*CL+256) -> [128, HL*C]: partition p, col
        # block h*C+o = WoT[m*CL + h*128 + p, o]
        wo_loc = np.ascontiguousarray(
            WoT[sl, :].reshape(HL, 128, C).transpose(1, 0, 2).reshape(
                128, HL * C))
        in_maps.append({
            "xt": xT,
            "wqT": conv(wlay(np.asarray(Wq)[sl, :].T, CL)),
            "wkT": conv(wlay(np.asarray(Wk)[sl, :].T, CL)),
            "wvT": conv(wlay(np.asarray(Wv)[sl, :].T, CL)),
            "woT": conv(wo_loc),
        })
    return in_maps


def kernel(x, Wq, Wk, Wv, Wo, _trace=False):
    in_maps = make_in_maps(x, Wq, Wk, Wv, Wo)
    nc = _get_nc()
    res = run_bass_kernel_spmd(nc, in_maps, list(range(NCORES)),
                               trace=_trace)
    acc = np.zeros((BT, C), dtype=np.float32)
    for m in range(NCORES):
        acc += res.results[m]["out"].astype(np.float32)
    out = acc.reshape(B, T, C)
    if _trace:
        return out, res
    return out


# revision 63
# speedup vs baseline: 1.1693x; 1.1693x over previous
"""Causal self-attention with RoPE (B=2, T=1024, C=2048, H=16) on 8 TRN2
NeuronCores, head-parallel tensor sharding (2 heads per core) with
COLLECTIVE-FREE partial-sum output.

Design:
  - x^T replicated as bf16 ExternalInput; tiles DMA'd straight to SBUF.
  - RoPE cos/sin tables ride as inline (Const) tensors baked into the NEFF.
  - QKV projections tensor-parallel (weights column-sharded, bf16); RoPE via
    PSUM-copy-first + SBUF partition-shift DMA + DVE mul/add.
  - Causal attention in [tk, tq] layout: exp on ScalarE straight out of
    PSUM, softmax denominator via an all-ones-lhsT matmul, unnormalized y
    accumulated in PSUM, one reciprocal + multiply. Fully-masked k-tiles
    skipped (c0 trimming).
  - NO collectives: per 512-token block, both local heads' attention runs
    back-to-back, then the output projection contracts ONLY this core's 256
    y-channels against its [256, 2048] slice of Wo, accumulating the two
    heads in PSUM. Each core emits a FULL-shape [2048, 2048] bf16 PARTIAL
    output; the host sums the 8 partials (f32) and reshapes. This replaces
    two AllToAlls (~50-60us each on HW) + full-Wo loads with zero on-device
    communication.
Host reassembles: sum core partials (f32), reshape to [B, T, C].
"""
import numpy as np

import concourse.bass as bass
import concourse.mybir as mybir
import concourse.tile as tile
from concourse import bacc
from concourse.bass_utils import run_bass_kernel_spmd

F32 = mybir.dt.float32
F32R = mybir.dt.float32r
BF16 = mybir.dt.bfloat16

B, T, C = 2, 1024, 2048
H = 16
D = C // H            # 128
BT = B * T            # 2048
NCORES = 8
HL = H // NCORES      # heads per core = 2
CL = HL * D           # local channels = 256
ATT_SCALE = 1.0 / float(np.sqrt(D))
ROPE_BASE = 10000.0
NEG = -1.0e30

CT = C // 128         # 16 contraction tiles
TB = BT // 512        # 4 token blocks of 512


def _rope_tables():
    inv_freq = 1.0 / (ROPE_BASE ** (np.arange(0, D, 2, dtype=np.float64) / D))
    t = np.arange(T, dtype=np.float64)
    freqs = np.outer(t, inv_freq)                        # [T, D/2]
    emb = np.concatenate([freqs, freqs], axis=-1)        # [T, D]
    cos = np.cos(emb).astype(np.float32)                 # [T, D]
    sin = np.sin(emb).astype(np.float32)
    cosT = np.ascontiguousarray(cos.T)                   # [D, T]
    sinT = np.ascontiguousarray(sin.T)
    # s[p] = q[p]*ssin[p]; rope = q*cos + shift64(s) needs ssin negated on
    # the SECOND half (s2[p<64] = s[p+64] must equal -q[p+64]*sin[p])
    sgn_sinT = sinT.copy()
    sgn_sinT[D // 2:] *= -1.0
    return cosT, sgn_sinT


def _build(variant="full"):
    # variant "smallout": 1MB output (timing experiment only, wrong results)
    smallout = variant == "smallout"
    nc = bacc.Bacc("TRN2", target_bir_lowering=False, debug=False,
                   num_devices=NCORES)

    # full x^T, replicated on every core
    xt_d = nc.dram_tensor("xt", [C, BT], BF16, kind="ExternalInput").ap()
    # qkv weights column-sharded, laid out [128, CT*CL]: partition p holds
    # WT[ct*128+p, o] at free offset ct*CL+o
    wqT_d = nc.dram_tensor("wqT", [128, CT * CL], BF16, kind="ExternalInput").ap()
    wkT_d = nc.dram_tensor("wkT", [128, CT * CL], BF16, kind="ExternalInput").ap()
    wvT_d = nc.dram_tensor("wvT", [128, CT * CL], BF16, kind="ExternalInput").ap()
    # this core's 256-row slice of Wo^T: col block h*C+o = WoT[my_c0+h*128+p, o]
    woT_d = nc.dram_tensor("woT", [128, HL * C], BF16, kind="ExternalInput").ap()
    # full-shape PARTIAL output (this core's 2 heads' contribution)
    out_d = nc.dram_tensor("out", [256 if smallout else BT, C], BF16,
                           kind="ExternalOutput").ap()

    # RoPE tables baked into the NEFF (loaded to HBM at model-load time)
    import ml_dtypes
    cosT, sgn_sinT = _rope_tables()
    cos_d = nc.inline_tensor(cosT.astype(ml_dtypes.bfloat16),
                             name="ropecos")             # [128, T] bf16
    sin_d = nc.inline_tensor(sgn_sinT.astype(ml_dtypes.bfloat16),
                             name="ropesin")             # [128, T] bf16

    with tile.TileContext(nc) as tc:
        with (
            tc.tile_pool(name="wpool", bufs=1) as wpool,
            tc.tile_pool(name="const", bufs=1) as cpool,
            tc.tile_pool(name="qkv", bufs=1) as qkvpool,
            tc.tile_pool(name="xs", bufs=6) as xspool,
            tc.tile_pool(name="rope", bufs=1) as ropepool,
            tc.tile_pool(name="att", bufs=3) as attpool,
            tc.tile_pool(name="ob", bufs=6) as obpool,
        ):
            # ---- startup loads, interleaved per queue so the first weight
            # chunks + first x tile land ~2us in and PE starts immediately;
            # cos/sin/wo (needed later) ride behind the first token block ----
            # ones first, on DVE only (no queue DMAs ahead of it), so the
            # p-state warm matmuls can start at t~0.4us
            ones_f = cpool.tile([128, 128], F32, tag="onesf")
            nc.vector.memset(ones_f[:], 1.0)
            ones_sb = cpool.tile([128, 128], BF16, tag="ones")
            nc.vector.tensor_copy(ones_sb[:], ones_f[:])
            ones_w = cpool.tile([128, 512], BF16, tag="onesw")
            nc.vector.memset(ones_w[:], 1.0)
            # warm the Exp table so LoadActFuncSet isn't on the QKV->att
            # critical path
            warm = cpool.tile([128, 128], F32, tag="warm")
            nc.scalar.activation(warm[:], ones_f[:],
                                 mybir.ActivationFunctionType.Exp)

            wq_sb = wpool.tile([128, CT * CL], BF16, tag="wq")
            wk_sb = wpool.tile([128, CT * CL], BF16, tag="wk")
            wv_sb = wpool.tile([128, CT * CL], BF16, tag="wv")
            qs = (nc.sync, nc.scalar, nc.gpsimd)
            wsrc = ((wq_sb, wqT_d), (wk_sb, wkT_d), (wv_sb, wvT_d))
            WCH = 4                       # cts per weight chunk
            cw = WCH * CL
            # x loaded as [128, 1024] tiles (per batch), ALL issued upfront:
            # big transfers amortize the ~1us per-DMA queue overhead, and
            # full SBUF residency (64KB/partition) kills every later x wait.
            # Weight chunks are graduated (1,3,4,8 cts) so the very first
            # matmul's deps are tiny and early x tiles interleave tightly.
            xs0_tiles = []
            xi = 0
            for c in range(CT // WCH):
                for qi, (w_sb, w_d) in enumerate(wsrc):
                    qs[qi].dma_start(out=w_sb[:, c * cw:(c + 1) * cw],
                                     in_=w_d[:, c * cw:(c + 1) * cw])
                for _ in range(3):
                    xs = xspool.tile([128, 512], BF16, tag="xs", bufs=24,
                                     name=f"xs0_{xi}")
                    qs[xi % 3].dma_start(
                        out=xs[:], in_=xt_d[xi * 128:(xi + 1) * 128, 0:512])
                    xs0_tiles.append(xs)
                    xi += 1
            while xi < CT:
                xs = xspool.tile([128, 512], BF16, tag="xs", bufs=24,
                                 name=f"xs0_{xi}")
                qs[xi % 3].dma_start(
                    out=xs[:], in_=xt_d[xi * 128:(xi + 1) * 128, 0:512])
                xs0_tiles.append(xs)
                xi += 1

            cos_sb = cpool.tile([D, T], BF16, tag="cos")
            sin_sb = cpool.tile([D, T], BF16, tag="sin")
            nc.scalar.dma_start(out=cos_sb[:], in_=cos_d.ap())
            nc.sync.dma_start(out=sin_sb[:], in_=sin_d.ap())

            # additive causal mask for diagonal 128x128 blocks:
            # rows=tk, cols=tq; keep (0.0) where tk <= tq else NEG
            mask_sb = cpool.tile([128, 128], F32, tag="mask")
            nc.gpsimd.memset(mask_sb[:], 0.0)
            nc.gpsimd.affine_select(
                out=mask_sb[:], in_=mask_sb[:],
                compare_op=mybir.AluOpType.is_ge,
                fill=NEG, base=0,
                pattern=[[1, 128]], channel_multiplier=-1,
            )

            # my 256-row Wo^T slice (bf16, 8KB/partition)
            wo_sb = wpool.tile([128, HL * C], BF16, tag="wo")
            nc.gpsimd.dma_start(out=wo_sb[:], in_=woT_d)

            # persistent qkv activations, split per batch so attention on
            # batch 0 doesn't false-depend on batch-1 rope writes
            qT = [[qkvpool.tile([D, T], BF16, tag=f"qT{h}{b}", name=f"qT{h}{b}")
                   for b in range(B)] for h in range(HL)]
            kT = [[qkvpool.tile([D, T], BF16, tag=f"kT{h}{b}", name=f"kT{h}{b}")
                   for b in range(B)] for h in range(HL)]
            v_sb = [qkvpool.tile([128, (T // 128) * CL], BF16, tag=f"v{b}",
                                 name=f"v{b}")
                    for b in range(B)]

            # ---- phase 1: QKV projections + rope ----
            with tc.tile_pool(name="psqkv", bufs=1, space="PSUM") as psq:
                # dummy matmuls on ones (no DMA deps) ramp the PE p-state
                # to full clock while the first weight/x DMAs land; they
                # borrow the pv0 bank (bufs=1: same bank as the real pv0)
                warm_ps = psq.tile([128, 512], F32, tag="pv0", name="warm")
                for _ in range(24):
                    nc.tensor.matmul(warm_ps[:, 0:128], ones_sb[:],
                                     ones_sb[:], start=True, stop=True)
                for tb in range(TB):
                    tcol = tb * 512
                    rcol = tcol % T          # rope table column (per batch)
                    ps_q = [psq.tile([128, 512], F32, tag=f"pq{h}", name=f"pq{h}")
                            for h in range(HL)]
                    ps_k = [psq.tile([128, 512], F32, tag=f"pk{h}", name=f"pk{h}")
                            for h in range(HL)]
                    ps_v = [psq.tile([128, CL], F32, tag=f"pv{i}", name=f"pv{i}")
                            for i in range(4)]
                    for ct in range(CT):
                        if tb == 0:
                            xs = xs0_tiles[ct]
                        else:
                            xs = xspool.tile([128, 512], BF16, tag="xs",
                                             bufs=24)
                            eng = (nc.sync, nc.scalar, nc.gpsimd)[ct % 3]
                            eng.dma_start(
                                out=xs[:],
                                in_=xt_d[ct * 128:(ct + 1) * 128,
                                         tcol:tcol + 512],
                            )
                        st, sp = ct == 0, ct == CT - 1
                        for h in range(HL):
                            nc.tensor.matmul(
                                ps_q[h][:],
                                wq_sb[:, ct * CL + h * D: ct * CL + (h + 1) * D],
                                xs[:], start=st, stop=sp)
                            nc.tensor.matmul(
                                ps_k[h][:],
                                wk_sb[:, ct * CL + h * D: ct * CL + (h + 1) * D],
                                xs[:], start=st, stop=sp)
                        for i in range(4):
                            nc.tensor.matmul(
                                ps_v[i][:],
                                xs[:, i * 128:(i + 1) * 128],
                                wv_sb[:, ct * CL:(ct + 1) * CL],
                                start=st, stop=sp)
                    # Drain ALL psum banks first: 4 rope staging copies (h=0
                    # on DVE, h=1 on ActE — GpSimd cannot read PSUM), then v
                    # copies on ActE. Rope math afterwards reads the copies:
                    # s = tmp*ssin, shift64 via two HWDGE DMAs, dst =
                    # tmp*cos + shift64(s). h=0 math on DVE, h=1 on Pool.
                    bb = tb // 2
                    staged = []
                    for h in range(HL):
                        for ps, dst in ((ps_q[h], qT[h][bb]),
                                        (ps_k[h], kT[h][bb])):
                            tmp = ropepool.tile([128, 512], BF16, tag="rtmp",
                                                bufs=4)
                            if h == 0:
                                nc.vector.tensor_copy(tmp[:], ps[:])
                            else:
                                nc.scalar.activation(
                                    tmp[:], ps[:],
                                    mybir.ActivationFunctionType.Copy)
                            staged.append((tmp, dst))
                    for i in range(4):
                        gt = (tb % 2) * 4 + i
                        nc.scalar.activation(
                            v_sb[bb][:, gt * CL:(gt + 1) * CL], ps_v[i][:],
                            mybir.ActivationFunctionType.Copy)
                    # rope math in bf16 at 2x DVE rate
                    for tmp, dst in staged:
                        s = ropepool.tile([128, 512], BF16, tag="rs",
                                          bufs=4)
                        nc.vector.tensor_mul(
                            s[:], tmp[:], sin_sb[:, rcol:rcol + 512])
                        rot = ropepool.tile([128, 512], BF16, tag="rrot",
                                            bufs=4)
                        nc.sync.dma_start(out=rot[0:64, :],
                                          in_=s[64:128, :])
                        nc.scalar.dma_start(out=rot[64:128, :],
                                            in_=s[0:64, :])
                        u = ropepool.tile([128, 512], BF16, tag="ru",
                                          bufs=4)
                        nc.vector.tensor_mul(
                            u[:], tmp[:], cos_sb[:, rcol:rcol + 512])
                        nc.vector.tensor_add(
                            dst[:, rcol:rcol + 512], u[:], rot[:])

            # ---- phase 2: attention (block-outer, head-inner) + fused
            # partial output projection, pipelined one block behind so proj
            # never stalls on the reciprocal chain. The j-loop issues scores
            # two tiles ahead of the l/y matmuls so the in-order PE queue
            # rides out the exp-chain latency. ----
            def emit_proj(blk):
                b, jj, yb = blk
                row0 = 0 if smallout else b * T + jj * 512
                for tc_ in range(4):
                    for oq in range(4):
                        ps_o = psa.tile([128, 512], F32, tag="o", bufs=2)
                        for h in range(HL):
                            nc.tensor.matmul(
                                ps_o[:],
                                yb[h][:, tc_ * 128:(tc_ + 1) * 128],
                                wo_sb[:, h * C + oq * 512:
                                      h * C + (oq + 1) * 512],
                                start=(h == 0), stop=(h == HL - 1))
                        # ob copies alternate DVE/Act; out-DMAs on
                        # sync/gpsimd (Act queue mostly stays with exps)
                        ob = obpool.tile([128, 512], BF16, tag="ob")
                        if oq % 2 == 0:
                            nc.vector.tensor_copy(ob[:], ps_o[:])
                        else:
                            nc.scalar.activation(
                                ob[:], ps_o[:],
                                mybir.ActivationFunctionType.Copy)
                        r0 = row0 + (tc_ % 2 if smallout else tc_) * 128
                        eng = (nc.sync, nc.gpsimd)[oq % 2]
                        eng.dma_start(
                            out=out_d[r0:r0 + 128,
                                      oq * 512:(oq + 1) * 512],
                            in_=ob[:])

            with tc.tile_pool(name="psatt", bufs=1, space="PSUM") as psa:
                # keep the PE clock hot across the QKV->attention drain
                warm2 = psa.tile([128, 512], F32, tag="o", bufs=2,
                                 name="warm2")
                for _ in range(8):
                    nc.tensor.matmul(warm2[:], ones_sb[:],
                                     ones_w[:], start=True, stop=True)
                prev_blk = None
                for b, jj in ((0, 0), (1, 0), (0, 1), (1, 1)):
                    lcol = jj * 512
                    njt = 4 * jj + 4
                    yb = [None, None]
                    ps_y = [psa.tile([128, 512], F32, tag=f"y{h}", bufs=1,
                                     name=f"y{h}") for h in range(HL)]
                    ps_l = [psa.tile([128, 512], F32, tag=f"l{h}", bufs=1,
                                     name=f"l{h}") for h in range(HL)]

                    def flush(ent):
                        h, j, p, c0 = ent
                        st, sp = j == 0, j == njt - 1
                        nc.tensor.matmul(
                            ps_l[h][:, c0:512], ones_sb[:],
                            p[:, c0:512], start=st, stop=sp)
                        nc.tensor.matmul(
                            ps_y[h][:, c0:512],
                            v_sb[b][:, j * CL + h * D:
                                     j * CL + (h + 1) * D],
                            p[:, c0:512], start=st, stop=sp)

                    # heads interleaved: each head's exp latency hides
                    # behind the other head's scores + l/y matmuls
                    pend = []
                    for j in range(njt):
                        c0 = max(0, j * 128 - jj * 512)
                        diag0 = j * 128 - jj * 512
                        for h in range(HL):
                            ps_s = psa.tile([128, 512], F32, tag="s",
                                            bufs=2)
                            nc.tensor.matmul(
                                ps_s[:, c0:512],
                                kT[h][b][:, j * 128:(j + 1) * 128],
                                qT[h][b][:, lcol + c0: lcol + 512],
                                start=True, stop=True)
                            if 0 <= diag0 < 512:
                                nc.vector.tensor_add(
                                    ps_s[:, diag0:diag0 + 128],
                                    ps_s[:, diag0:diag0 + 128],
                                    mask_sb[:])
                            p = attpool.tile([128, 512], BF16, tag="p",
                                             bufs=4)
                            nc.scalar.activation(
                                p[:, c0:512], ps_s[:, c0:512],
                                mybir.ActivationFunctionType.Exp,
                                scale=ATT_SCALE)
                            pend.append((h, j, p, c0))
                            if len(pend) > 2:
                                flush(pend.pop(0))
                    for ent in pend:
                        flush(ent)
                    for h in range(HL):
                        rec = attpool.tile([128, 512], F32, tag="rec",
                                           bufs=2)
                        nc.vector.reciprocal(rec[:], ps_l[h][:])
                        # normalized y in bf16: [128 y-chans of h, 512 tok]
                        yb[h] = attpool.tile([128, 512], BF16, tag=f"yb{h}",
                                             bufs=2, name=f"yb{h}")
                        nc.vector.tensor_mul(yb[h][:], ps_y[h][:], rec[:])
                    # prev block's proj AFTER rec/yb so those get DVE queue
                    # priority; its matmuls fill PE while rec/yb drain
                    if prev_blk is not None:
                        emit_proj(prev_blk)
                    prev_blk = (b, jj, yb)
                emit_proj(prev_blk)

    nc.compile()
    return nc


_NC_CACHE = None


def _get_nc():
    global _NC_CACHE
    if _NC_CACHE is None:
        _NC_CACHE = _build()
    return _NC_CACHE


def make_in_maps(x, Wq, Wk, Wv, Wo):
    import ml_dtypes

    def conv(a):
        return np.ascontiguousarray(a).astype(ml_dtypes.bfloat16)

    x = np.asarray(x, dtype=np.float32)
    xT = conv(x.reshape(BT, C).T)                        # [C, BT] bf16

    def wlay(wT, cols):
        # [C, cols] -> [128, CT*cols]: partition p holds WT[ct*128+p, :]
        return np.ascontiguousarray(
            wT.reshape(CT, 128, cols).transpose(1, 0, 2).reshape(
                128, CT * cols))

    WoT = np.asarray(Wo, dtype=np.float32).T             # [C, C]
    in_maps = []
    for m in range(NCORES):
        sl = slice(m * CL, (m + 1) * CL)
        # my Wo^T rows [m*CL, m*CL+256) -> [128, HL*C]: partition p, col
        # block h*C+o = WoT[m*CL + h*128 + p, o]
        wo_loc = np.ascontiguousarray(
            WoT[sl, :].reshape(HL, 128, C).transpose(1, 0, 2).reshape(
                128, HL * C))
        in_maps.append({
            "xt": xT,
            "wqT": conv(wlay(np.asarray(Wq)[sl, :].T, CL)),
            "wkT": conv(wlay(np.asarray(Wk)[sl, :].T, CL)),
            "wvT": conv(wlay(np.asarray(Wv)[sl, :].T, CL)),
            "woT": conv(wo_loc),
        })
    return in_maps


def kernel(x, Wq, Wk, Wv, Wo, _trace=False):
    in_maps = make_in_maps(x, Wq, Wk, Wv, Wo)
    nc = _get_nc()
    res = run_bass_kernel_spmd(nc, in_maps, list(range(NCORES)),
                               trace=_trace)
    acc = np.zeros((BT, C), dtype=np.float32)
    for m in range(NCORES):
        acc += res.results[m]["out"].astype(np.float32)
    out = acc.reshape(B, T, C)
    if _trace:
        return out, res
    return out


# revision 67
# speedup vs baseline: 1.2013x; 1.0273x over previous
"""Causal self-attention with RoPE (B=2, T=1024, C=2048, H=16) on 8 TRN2
NeuronCores, head-parallel tensor sharding (2 heads per core) with
COLLECTIVE-FREE partial-sum output.

Design:
  - x^T replicated as bf16 ExternalInput; tiles DMA'd straight to SBUF.
  - RoPE cos/sin tables ride as inline (Const) tensors baked into the NEFF.
  - QKV projections tensor-parallel (weights column-sharded, bf16); RoPE via
    PSUM-copy-first + SBUF partition-shift DMA + DVE mul/add.
  - Causal attention in [tk, tq] layout: exp on ScalarE straight out of
    PSUM, softmax denominator via an all-ones-lhsT matmul, unnormalized y
    accumulated in PSUM, one reciprocal + multiply. Fully-masked k-tiles
    skipped (c0 trimming).
  - NO collectives: per 512-token block, both local heads' attention runs
    back-to-back, then the output projection contracts ONLY this core's 256
    y-channels against its [256, 2048] slice of Wo, accumulating the two
    heads in PSUM. Each core emits a FULL-shape [2048, 2048] bf16 PARTIAL
    output; the host sums the 8 partials (f32) and reshapes. This replaces
    two AllToAlls (~50-60us each on HW) + full-Wo loads with zero on-device
    communication.
Host reassembles: sum core partials (f32), reshape to [B, T, C].
"""
import numpy as np

import concourse.bass as bass
import concourse.mybir as mybir
import concourse.tile as tile
from concourse import bacc
from concourse.bass_utils import run_bass_kernel_spmd

F32 = mybir.dt.float32
F32R = mybir.dt.float32r
BF16 = mybir.dt.bfloat16

B, T, C = 2, 1024, 2048
H = 16
D = C // H            # 128
BT = B * T            # 2048
NCORES = 8
HL = H // NCORES      # heads per core = 2
CL = HL * D           # local channels = 256
ATT_SCALE = 1.0 / float(np.sqrt(D))
ROPE_BASE = 10000.0
NEG = -1.0e30

CT = C // 128         # 16 contraction tiles
TB = BT // 512        # 4 token blocks of 512


def _rope_tables():
    inv_freq = 1.0 / (ROPE_BASE ** (np.arange(0, D, 2, dtype=np.float64) / D))
    t = np.arange(T, dtype=np.float64)
    freqs = np.outer(t, inv_freq)                        # [T, D/2]
    emb = np.concatenate([freqs, freqs], axis=-1)        # [T, D]
    cos = np.cos(emb).astype(np.float32)                 # [T, D]
    sin = np.sin(emb).astype(np.float32)
    cosT = np.ascontiguousarray(cos.T)                   # [D, T]
    sinT = np.ascontiguousarray(sin.T)
    # s[p] = q[p]*ssin[p]; rope = q*cos + shift64(s) needs ssin negated on
    # the SECOND half (s2[p<64] = s[p+64] must equal -q[p+64]*sin[p])
    sgn_sinT = sinT.copy()
    sgn_sinT[D // 2:] *= -1.0
    return cosT, sgn_sinT


def _build(variant="full"):
    # variant "smallout": 1MB output (timing experiment only, wrong results)
    smallout = variant == "smallout"
    nc = bacc.Bacc("TRN2", target_bir_lowering=False, debug=False,
                   num_devices=NCORES)

    # full x^T, replicated on every core
    xt_d = nc.dram_tensor("xt", [C, BT], BF16, kind="ExternalInput").ap()
    # qkv weights column-sharded, laid out [128, CT*CL]: partition p holds
    # WT[ct*128+p, o] at free offset ct*CL+o
    wqT_d = nc.dram_tensor("wqT", [128, CT * CL], BF16, kind="ExternalInput").ap()
    wkT_d = nc.dram_tensor("wkT", [128, CT * CL], BF16, kind="ExternalInput").ap()
    wvT_d = nc.dram_tensor("wvT", [128, CT * CL], BF16, kind="ExternalInput").ap()
    # this core's 256-row slice of Wo^T: col block h*C+o = WoT[my_c0+h*128+p, o]
    woT_d = nc.dram_tensor("woT", [128, HL * C], BF16, kind="ExternalInput").ap()
    # full-shape PARTIAL output (this core's 2 heads' contribution)
    out_d = nc.dram_tensor("out", [256 if smallout else BT, C], BF16,
                           kind="ExternalOutput").ap()

    # RoPE tables baked into the NEFF (loaded to HBM at model-load time)
    import ml_dtypes
    cosT, sgn_sinT = _rope_tables()
    cos_d = nc.inline_tensor(cosT.astype(ml_dtypes.bfloat16),
                             name="ropecos")             # [128, T] bf16
    sin_d = nc.inline_tensor(sgn_sinT.astype(ml_dtypes.bfloat16),
                             name="ropesin")             # [128, T] bf16

    with tile.TileContext(nc) as tc:
        with (
            tc.tile_pool(name="wpool", bufs=1) as wpool,
            tc.tile_pool(name="const", bufs=1) as cpool,
            tc.tile_pool(name="qkv", bufs=1) as qkvpool,
            tc.tile_pool(name="xs", bufs=6) as xspool,
            tc.tile_pool(name="rope", bufs=1) as ropepool,
            tc.tile_pool(name="att", bufs=3) as attpool,
            tc.tile_pool(name="ob", bufs=6) as obpool,
        ):
            # ---- startup loads, interleaved per queue so the first weight
            # chunks + first x tile land ~2us in and PE starts immediately;
            # cos/sin/wo (needed later) ride behind the first token block ----
            # ones first, on DVE only (no queue DMAs ahead of it), so the
            # p-state warm matmuls can start at t~0.4us
            ones_f = cpool.tile([128, 128], F32, tag="onesf")
            nc.vector.memset(ones_f[:], 1.0)
            ones_sb = cpool.tile([128, 128], BF16, tag="ones")
            nc.vector.tensor_copy(ones_sb[:], ones_f[:])
            ones_w = cpool.tile([128, 512], BF16, tag="onesw")
            nc.vector.memset(ones_w[:], 1.0)
            # warm the Exp table so LoadActFuncSet isn't on the QKV->att
            # critical path
            warm = cpool.tile([128, 128], F32, tag="warm")
            nc.scalar.activation(warm[:], ones_f[:],
                                 mybir.ActivationFunctionType.Exp)

            wq_sb = wpool.tile([128, CT * CL], BF16, tag="wq")
            wk_sb = wpool.tile([128, CT * CL], BF16, tag="wk")
            wv_sb = wpool.tile([128, CT * CL], BF16, tag="wv")
            qs = (nc.sync, nc.scalar, nc.gpsimd)
            wsrc = ((wq_sb, wqT_d), (wk_sb, wkT_d), (wv_sb, wvT_d))
            WCH = 4                       # cts per weight chunk
            cw = WCH * CL
            # x loaded as [128, 1024] tiles (per batch), ALL issued upfront:
            # big transfers amortize the ~1us per-DMA queue overhead, and
            # full SBUF residency (64KB/partition) kills every later x wait.
            # Weight chunks are graduated (1,3,4,8 cts) so the very first
            # matmul's deps are tiny and early x tiles interleave tightly.
            xs0_tiles = []
            xi = 0
            for c in range(CT // WCH):
                for qi, (w_sb, w_d) in enumerate(wsrc):
                    qs[qi].dma_start(out=w_sb[:, c * cw:(c + 1) * cw],
                                     in_=w_d[:, c * cw:(c + 1) * cw])
                for _ in range(3):
                    xs = xspool.tile([128, 512], BF16, tag="xs", bufs=24,
                                     name=f"xs0_{xi}")
                    qs[xi % 3].dma_start(
                        out=xs[:], in_=xt_d[xi * 128:(xi + 1) * 128, 0:512])
                    xs0_tiles.append(xs)
                    xi += 1
            while xi < CT:
                xs = xspool.tile([128, 512], BF16, tag="xs", bufs=24,
                                 name=f"xs0_{xi}")
                qs[xi % 3].dma_start(
                    out=xs[:], in_=xt_d[xi * 128:(xi + 1) * 128, 0:512])
                xs0_tiles.append(xs)
                xi += 1

            cos_sb = cpool.tile([D, T], BF16, tag="cos")
            sin_sb = cpool.tile([D, T], BF16, tag="sin")
            nc.scalar.dma_start(out=cos_sb[:], in_=cos_d.ap())
            nc.sync.dma_start(out=sin_sb[:], in_=sin_d.ap())

            # additive causal mask for diagonal 128x128 blocks:
            # rows=tk, cols=tq; keep (0.0) where tk <= tq else NEG
            mask_sb = cpool.tile([128, 128], F32, tag="mask")
            nc.gpsimd.memset(mask_sb[:], 0.0)
            nc.gpsimd.affine_select(
                out=mask_sb[:], in_=mask_sb[:],
                compare_op=mybir.AluOpType.is_ge,
                fill=NEG, base=0,
                pattern=[[1, 128]], channel_multiplier=-1,
            )

            # my 256-row Wo^T slice (bf16, 8KB/partition)
            wo_sb = wpool.tile([128, HL * C], BF16, tag="wo")
            nc.gpsimd.dma_start(out=wo_sb[:], in_=woT_d)

            # persistent qkv activations, split per batch so attention on
            # batch 0 doesn't false-depend on batch-1 rope writes
            qT = [[qkvpool.tile([D, T], BF16, tag=f"qT{h}{b}", name=f"qT{h}{b}")
                   for b in range(B)] for h in range(HL)]
            kT = [[qkvpool.tile([D, T], BF16, tag=f"kT{h}{b}", name=f"kT{h}{b}")
                   for b in range(B)] for h in range(HL)]
            v_sb = [qkvpool.tile([128, (T // 128) * CL], BF16, tag=f"v{b}",
                                 name=f"v{b}")
                    for b in range(B)]

            # ---- phase 1: QKV projections + rope ----
            with tc.tile_pool(name="psqkv", bufs=1, space="PSUM") as psq:
                # dummy matmuls on ones (no DMA deps) ramp the PE p-state
                # to full clock while the first weight/x DMAs land; they
                # borrow the pv0 bank (bufs=1: same bank as the real pv0)
                warm_ps = psq.tile([128, 512], F32, tag="pv0", name="warm")
                for _ in range(24):
                    nc.tensor.matmul(warm_ps[:, 0:128], ones_sb[:],
                                     ones_sb[:], start=True, stop=True)
                for tb in range(TB):
                    tcol = tb * 512
                    rcol = tcol % T          # rope table column (per batch)
                    ps_q = [psq.tile([128, 512], F32, tag=f"pq{h}", name=f"pq{h}")
                            for h in range(HL)]
                    ps_k = [psq.tile([128, 512], F32, tag=f"pk{h}", name=f"pk{h}")
                            for h in range(HL)]
                    ps_v = [psq.tile([128, CL], F32, tag=f"pv{i}", name=f"pv{i}")
                            for i in range(4)]
                    for ct in range(CT):
                        if tb == 0:
                            xs = xs0_tiles[ct]
                        else:
                            xs = xspool.tile([128, 512], BF16, tag="xs",
                                             bufs=24)
                            eng = (nc.sync, nc.scalar, nc.gpsimd)[ct % 3]
                            eng.dma_start(
                                out=xs[:],
                                in_=xt_d[ct * 128:(ct + 1) * 128,
                                         tcol:tcol + 512],
                            )
                        st, sp = ct == 0, ct == CT - 1
                        for h in range(HL):
                            nc.tensor.matmul(
                                ps_q[h][:],
                                wq_sb[:, ct * CL + h * D: ct * CL + (h + 1) * D],
                                xs[:], start=st, stop=sp)
                            nc.tensor.matmul(
                                ps_k[h][:],
                                wk_sb[:, ct * CL + h * D: ct * CL + (h + 1) * D],
                                xs[:], start=st, stop=sp)
                        for i in range(4):
                            nc.tensor.matmul(
                                ps_v[i][:],
                                xs[:, i * 128:(i + 1) * 128],
                                wv_sb[:, ct * CL:(ct + 1) * CL],
                                start=st, stop=sp)
                    # Drain ALL psum banks first: 4 rope staging copies (h=0
                    # on DVE, h=1 on ActE — GpSimd cannot read PSUM), then v
                    # copies on ActE. Rope math afterwards reads the copies:
                    # s = tmp*ssin, shift64 via two HWDGE DMAs, dst =
                    # tmp*cos + shift64(s). h=0 math on DVE, h=1 on Pool.
                    bb = tb // 2
                    staged = []
                    for h in range(HL):
                        for ps, dst in ((ps_q[h], qT[h][bb]),
                                        (ps_k[h], kT[h][bb])):
                            tmp = ropepool.tile([128, 512], BF16, tag="rtmp",
                                                bufs=4)
                            if h == 0:
                                nc.vector.tensor_copy(tmp[:], ps[:])
                            else:
                                nc.scalar.activation(
                                    tmp[:], ps[:],
                                    mybir.ActivationFunctionType.Copy)
                            staged.append((tmp, dst))
                    for i in range(4):
                        gt = (tb % 2) * 4 + i
                        nc.scalar.activation(
                            v_sb[bb][:, gt * CL:(gt + 1) * CL], ps_v[i][:],
                            mybir.ActivationFunctionType.Copy)
                    # rope math in bf16 at 2x DVE rate
                    for tmp, dst in staged:
                        s = ropepool.tile([128, 512], BF16, tag="rs",
                                          bufs=4)
                        nc.vector.tensor_mul(
                            s[:], tmp[:], sin_sb[:, rcol:rcol + 512])
                        rot = ropepool.tile([128, 512], BF16, tag="rrot",
                                            bufs=4)
                        nc.sync.dma_start(out=rot[0:64, :],
                                          in_=s[64:128, :])
                        nc.scalar.dma_start(out=rot[64:128, :],
                                            in_=s[0:64, :])
                        u = ropepool.tile([128, 512], BF16, tag="ru",
                                          bufs=4)
                        nc.vector.tensor_mul(
                            u[:], tmp[:], cos_sb[:, rcol:rcol + 512])
                        nc.vector.tensor_add(
                            dst[:, rcol:rcol + 512], u[:], rot[:])

            # ---- phase 2: attention (block-outer, head-inner) + fused
            # partial output projection, pipelined one block behind so proj
            # never stalls on the reciprocal chain. The j-loop issues scores
            # two tiles ahead of the l/y matmuls so the in-order PE queue
            # rides out the exp-chain latency. ----
            def emit_proj(blk):
                b, jj, yb = blk
                row0 = 0 if smallout else b * T + jj * 512
                for tc_ in range(4):
                    for oq in range(4):
                        ps_o = psa.tile([128, 512], F32, tag="o", bufs=2)
                        for h in range(HL):
                            nc.tensor.matmul(
                                ps_o[:],
                                yb[h][:, tc_ * 128:(tc_ + 1) * 128],
                                wo_sb[:, h * C + oq * 512:
                                      h * C + (oq + 1) * 512],
                                start=(h == 0), stop=(h == HL - 1))
                        # ob copies alternate DVE/Act; out-DMAs on
                        # sync/gpsimd (Act queue mostly stays with exps)
                        ob = obpool.tile([128, 512], BF16, tag="ob")
                        if oq % 2 == 0:
                            nc.vector.tensor_copy(ob[:], ps_o[:])
                        else:
                            nc.scalar.activation(
                                ob[:], ps_o[:],
                                mybir.ActivationFunctionType.Copy)
                        r0 = row0 + (tc_ % 2 if smallout else tc_) * 128
                        eng = (nc.sync, nc.gpsimd)[oq % 2]
                        eng.dma_start(
                            out=out_d[r0:r0 + 128,
                                      oq * 512:(oq + 1) * 512],
                            in_=ob[:])

            with tc.tile_pool(name="psatt", bufs=1, space="PSUM") as psa:
                # keep the PE clock hot across the QKV->attention drain
                warm2 = psa.tile([128, 512], F32, tag="o", bufs=2,
                                 name="warm2")
                for _ in range(8):
                    nc.tensor.matmul(warm2[:], ones_sb[:],
                                     ones_w[:], start=True, stop=True)
                prev_blk = None
                for b, jj in ((0, 0), (1, 0), (0, 1), (1, 1)):
                    lcol = jj * 512
                    njt = 4 * jj + 4
                    yb = [None, None]
                    ps_y = [psa.tile([128, 512], F32, tag=f"y{h}", bufs=1,
                                     name=f"y{h}") for h in range(HL)]
                    ps_l = [psa.tile([128, 512], F32, tag=f"l{h}", bufs=1,
                                     name=f"l{h}") for h in range(HL)]

                    def flush(ent):
                        h, j, p, c0 = ent
                        st, sp = j == 0, j == njt - 1
                        nc.tensor.matmul(
                            ps_l[h][:, c0:512], ones_sb[:],
                            p[:, c0:512], start=st, stop=sp)
                        nc.tensor.matmul(
                            ps_y[h][:, c0:512],
                            v_sb[b][:, j * CL + h * D:
                                     j * CL + (h + 1) * D],
                            p[:, c0:512], start=st, stop=sp)

                    # heads interleaved: each head's exp latency hides
                    # behind the other head's scores + l/y matmuls
                    pend = []
                    for j in range(njt):
                        c0 = max(0, j * 128 - jj * 512)
                        diag0 = j * 128 - jj * 512
                        for h in range(HL):
                            ps_s = psa.tile([128, 512], F32, tag="s",
                                            bufs=2)
                            nc.tensor.matmul(
                                ps_s[:, c0:512],
                                kT[h][b][:, j * 128:(j + 1) * 128],
                                qT[h][b][:, lcol + c0: lcol + 512],
                                start=True, stop=True)
                            if 0 <= diag0 < 512:
                                nc.vector.tensor_add(
                                    ps_s[:, diag0:diag0 + 128],
                                    ps_s[:, diag0:diag0 + 128],
                                    mask_sb[:])
                            p = attpool.tile([128, 512], BF16, tag="p",
                                             bufs=4)
                            nc.scalar.activation(
                                p[:, c0:512], ps_s[:, c0:512],
                                mybir.ActivationFunctionType.Exp,
                                scale=ATT_SCALE)
                            pend.append((h, j, p, c0))
                            if len(pend) > 2:
                                flush(pend.pop(0))
                    for ent in pend:
                        flush(ent)
                    for h in range(HL):
                        rec = attpool.tile([128, 512], F32, tag="rec",
                                           bufs=2)
                        nc.vector.reciprocal(rec[:], ps_l[h][:])
                        # normalized y in bf16: [128 y-chans of h, 512 tok]
                        yb[h] = attpool.tile([128, 512], BF16, tag=f"yb{h}",
                                             bufs=2, name=f"yb{h}")
                        nc.vector.tensor_mul(yb[h][:], ps_y[h][:], rec[:])
                    # prev block's proj AFTER rec/yb so those get DVE queue
                    # priority; its matmuls fill PE while rec/yb drain
                    if prev_blk is not None:
                        emit_proj(prev_blk)
                    prev_blk = (b, jj, yb)
                emit_proj(prev_blk)

    nc.compile()
    return nc


_NC_CACHE = None


def _get_nc():
    global _NC_CACHE
    if _NC_CACHE is None:
        _NC_CACHE = _build()
    return _NC_CACHE


def make_in_maps(x, Wq, Wk, Wv, Wo):
    import ml_dtypes

    def conv(a):
        return np.ascontiguousarray(a).astype(ml_dtypes.bfloat16)

    x = np.asarray(x, dtype=np.float32)
    xT = conv(x.reshape(BT, C).T)                        # [C, BT] bf16

    def wlay(wT, cols):
        # [C, cols] -> [128, CT*cols]: partition p holds WT[ct*128+p, :]
        return np.ascontiguousarray(
            wT.reshape(CT, 128, cols).transpose(1, 0, 2).reshape(
                128, CT * cols))

    WoT = np.asarray(Wo, dtype=np.float32).T             # [C, C]
    in_maps = []
    for m in range(NCORES):
        sl = slice(m * CL, (m + 1) * CL)
        # my Wo^T rows [m*CL, m*CL+256) -> [128, HL*C]: partition p, col
        # block h*C+o = WoT[m*CL + h*128 + p, o]
        wo_loc = np.ascontiguousarray(
            WoT[sl, :].reshape(HL, 128, C).transpose(1, 0, 2).reshape(
                128, HL * C))
        in_maps.append({
            "xt": xT,
            "wqT": conv(wlay(np.asarray(Wq)[sl, :].T, CL)),
            "wkT": conv(wlay(np.asarray(Wk)[sl, :].T, CL)),
            "wvT": conv(wlay(np.asarray(Wv)[sl, :].T, CL)),
            "woT": conv(wo_loc),
        })
    return in_maps


def kernel(x, Wq, Wk, Wv, Wo, _trace=False):
    in_maps = make_in_maps(x, Wq, Wk, Wv, Wo)
    nc = _get_nc()
    res = run_bass_kernel_spmd(nc, in_maps, list(range(NCORES)),
                               trace=_trace)
    acc = np.zeros((BT, C), dtype=np.float32)
    for m in range(NCORES):
        acc += res.results[m]["out"].astype(np.float32)
    out = acc.reshape(B, T, C)
    if _trace:
        return out, res
    return out


# revision 68
# speedup vs baseline: 1.2996x; 1.0818x over previous
"""Causal self-attention with RoPE (B=2, T=1024, C=2048, H=16) on 8 TRN2
NeuronCores, head-parallel tensor sharding (2 heads per core) with
COLLECTIVE-FREE partial-sum output. TimelineSim: ~170us/core (v1
AllToAll design: 253us; measured HW deltas match).

Design:
  - NO collectives: per 512-token block, both local heads' attention runs
    back-to-back, then the output projection contracts ONLY this core's 256
    y-channels against its [256, 2048] slice of Wo, accumulating the two
    heads in PSUM. Each core emits a FULL-shape [2048, 2048] bf16 PARTIAL
    output; the host sums the 8 partials (f32) and reshapes. This replaces
    two AllToAlls (~50-60us each on HW) + full-Wo loads with zero on-device
    communication and zero cross-core skew sensitivity.
  - x^T replicated as bf16 ExternalInput; weight chunks + first-block x
    tiles interleaved across the 3 DMA queues so PE starts ~2us in.
  - Dummy ones-matmuls at phase starts ramp the PE p-state (0.65->2.4GHz
    needs ~3us of continuous work; each idle gap resets to 1.2GHz).
  - RoPE cos/sin tables (bf16) ride as inline tensors baked into the NEFF.
    Rope = tmp*cos + shift64(tmp*ssin) with the sign baked into ssin's
    second half-rows; staging copies split DVE/ActE (GpSimd cannot read
    PSUM), shift via two parallel HWDGE DMAs, math in bf16 at 2x DVE rate.
  - Causal attention in [tk, tq] layout, all operands bf16: exp on ScalarE
    straight out of PSUM, softmax denominator via an all-ones-lhsT matmul,
    unnormalized y accumulated in PSUM, one reciprocal + multiply.
    Fully-masked k-tiles skipped (c0 trimming). The two heads' j-loops are
    interleaved and scores are issued two tiles ahead of the l/y matmuls so
    the in-order PE queue rides out the exp-chain latency; the projection
    runs one block behind attention and its PSUM->bf16 copies are queued
    after rec/yb so the block-boundary chain never stalls.
Host reassembles: sum core partials (f32), reshape to [B, T, C].
"""
import numpy as np

import concourse.bass as bass
import concourse.mybir as mybir
import concourse.tile as tile
from concourse import bacc
from concourse.bass_utils import run_bass_kernel_spmd

F32 = mybir.dt.float32
F32R = mybir.dt.float32r
BF16 = mybir.dt.bfloat16

B, T, C = 2, 1024, 2048
H = 16
D = C // H            # 128
BT = B * T            # 2048
NCORES = 8
HL = H // NCORES      # heads per core = 2
CL = HL * D           # local channels = 256
ATT_SCALE = 1.0 / float(np.sqrt(D))
ROPE_BASE = 10000.0
NEG = -1.0e30

CT = C // 128         # 16 contraction tiles
TB = BT // 512        # 4 token blocks of 512


def _rope_tables():
    inv_freq = 1.0 / (ROPE_BASE ** (np.arange(0, D, 2, dtype=np.float64) / D))
    t = np.arange(T, dtype=np.float64)
    freqs = np.outer(t, inv_freq)                        # [T, D/2]
    emb = np.concatenate([freqs, freqs], axis=-1)        # [T, D]
    cos = np.cos(emb).astype(np.float32)                 # [T, D]
    sin = np.sin(emb).astype(np.float32)
    cosT = np.ascontiguousarray(cos.T)                   # [D, T]
    sinT = np.ascontiguousarray(sin.T)
    # s[p] = q[p]*ssin[p]; rope = q*cos + shift64(s) needs ssin negated on
    # the SECOND half (s2[p<64] = s[p+64] must equal -q[p+64]*sin[p])
    sgn_sinT = sinT.copy()
    sgn_sinT[D // 2:] *= -1.0
    return cosT, sgn_sinT


def _build(variant="full"):
    # variant "smallout": 1MB output (timing experiment only, wrong results)
    smallout = variant == "smallout"
    nc = bacc.Bacc("TRN2", target_bir_lowering=False, debug=False,
                   num_devices=NCORES)

    # full x^T, replicated on every core
    xt_d = nc.dram_tensor("xt", [C, BT], BF16, kind="ExternalInput").ap()
    # qkv weights column-sharded, laid out [128, CT*CL]: partition p holds
    # WT[ct*128+p, o] at free offset ct*CL+o
    wqT_d = nc.dram_tensor("wqT", [128, CT * CL], BF16, kind="ExternalInput").ap()
    wkT_d = nc.dram_tensor("wkT", [128, CT * CL], BF16, kind="ExternalInput").ap()
    wvT_d = nc.dram_tensor("wvT", [128, CT * CL], BF16, kind="ExternalInput").ap()
    # this core's 256-row slice of Wo^T: col block h*C+o = WoT[my_c0+h*128+p, o]
    woT_d = nc.dram_tensor("woT", [128, HL * C], BF16, kind="ExternalInput").ap()
    # full-shape PARTIAL output (this core's 2 heads' contribution)
    out_d = nc.dram_tensor("out", [256 if smallout else BT, C], BF16,
                           kind="ExternalOutput").ap()

    # RoPE tables baked into the NEFF (loaded to HBM at model-load time)
    import ml_dtypes
    cosT, sgn_sinT = _rope_tables()
    cos_d = nc.inline_tensor(cosT.astype(ml_dtypes.bfloat16),
                             name="ropecos")             # [128, T] bf16
    sin_d = nc.inline_tensor(sgn_sinT.astype(ml_dtypes.bfloat16),
                             name="ropesin")             # [128, T] bf16

    with tile.TileContext(nc) as tc:
        with (
            tc.tile_pool(name="wpool", bufs=1) as wpool,
            tc.tile_pool(name="const", bufs=1) as cpool,
            tc.tile_pool(name="qkv", bufs=1) as qkvpool,
            tc.tile_pool(name="xs", bufs=6) as xspool,
            tc.tile_pool(name="rope", bufs=1) as ropepool,
            tc.tile_pool(name="att", bufs=3) as attpool,
            tc.tile_pool(name="ob", bufs=6) as obpool,
        ):
            # ---- startup loads, interleaved per queue so the first weight
            # chunks + first x tile land ~2us in and PE starts immediately;
            # cos/sin/wo (needed later) ride behind the first token block ----
            # ones first, on DVE only (no queue DMAs ahead of it), so the
            # p-state warm matmuls can start at t~0.4us
            ones_f = cpool.tile([128, 128], F32, tag="onesf")
            nc.vector.memset(ones_f[:], 1.0)
            ones_sb = cpool.tile([128, 128], BF16, tag="ones")
            nc.vector.tensor_copy(ones_sb[:], ones_f[:])
            ones_w = cpool.tile([128, 512], BF16, tag="onesw")
            nc.vector.memset(ones_w[:], 1.0)
            # warm the Exp table so LoadActFuncSet isn't on the QKV->att
            # critical path
            warm = cpool.tile([128, 128], F32, tag="warm")
            nc.scalar.activation(warm[:], ones_f[:],
                                 mybir.ActivationFunctionType.Exp)

            wq_sb = wpool.tile([128, CT * CL], BF16, tag="wq")
            wk_sb = wpool.tile([128, CT * CL], BF16, tag="wk")
            wv_sb = wpool.tile([128, CT * CL], BF16, tag="wv")
            qs = (nc.sync, nc.scalar, nc.gpsimd)
            wsrc = ((wq_sb, wqT_d), (wk_sb, wkT_d), (wv_sb, wvT_d))
            WCH = 4                       # cts per weight chunk
            cw = WCH * CL
            # x loaded as [128, 1024] tiles (per batch), ALL issued upfront:
            # big transfers amortize the ~1us per-DMA queue overhead, and
            # full SBUF residency (64KB/partition) kills every later x wait.
            # Weight chunks are graduated (1,3,4,8 cts) so the very first
            # matmul's deps are tiny and early x tiles interleave tightly.
            xs0_tiles = []
            xi = 0
            for c in range(CT // WCH):
                for qi, (w_sb, w_d) in enumerate(wsrc):
                    qs[qi].dma_start(out=w_sb[:, c * cw:(c + 1) * cw],
                                     in_=w_d[:, c * cw:(c + 1) * cw])
                for _ in range(3):
                    xs = xspool.tile([128, 512], BF16, tag="xs", bufs=24,
                                     name=f"xs0_{xi}")
                    qs[xi % 3].dma_start(
                        out=xs[:], in_=xt_d[xi * 128:(xi + 1) * 128, 0:512])
                    xs0_tiles.append(xs)
                    xi += 1
            while xi < CT:
                xs = xspool.tile([128, 512], BF16, tag="xs", bufs=24,
                                 name=f"xs0_{xi}")
                qs[xi % 3].dma_start(
                    out=xs[:], in_=xt_d[xi * 128:(xi + 1) * 128, 0:512])
                xs0_tiles.append(xs)
                xi += 1

            cos_sb = cpool.tile([D, T], BF16, tag="cos")
            sin_sb = cpool.tile([D, T], BF16, tag="sin")
            nc.scalar.dma_start(out=cos_sb[:], in_=cos_d.ap())
            nc.sync.dma_start(out=sin_sb[:], in_=sin_d.ap())

            # additive causal mask for diagonal 128x128 blocks:
            # rows=tk, cols=tq; keep (0.0) where tk <= tq else NEG
            mask_sb = cpool.tile([128, 128], F32, tag="mask")
            nc.gpsimd.memset(mask_sb[:], 0.0)
            nc.gpsimd.affine_select(
                out=mask_sb[:], in_=mask_sb[:],
                compare_op=mybir.AluOpType.is_ge,
                fill=NEG, base=0,
                pattern=[[1, 128]], channel_multiplier=-1,
            )

            # my 256-row Wo^T slice (bf16, 8KB/partition)
            wo_sb = wpool.tile([128, HL * C], BF16, tag="wo")
            nc.gpsimd.dma_start(out=wo_sb[:], in_=woT_d)

            # persistent qkv activations, split per batch so attention on
            # batch 0 doesn't false-depend on batch-1 rope writes
            qT = [[qkvpool.tile([D, T], BF16, tag=f"qT{h}{b}", name=f"qT{h}{b}")
                   for b in range(B)] for h in range(HL)]
            kT = [[qkvpool.tile([D, T], BF16, tag=f"kT{h}{b}", name=f"kT{h}{b}")
                   for b in range(B)] for h in range(HL)]
            v_sb = [qkvpool.tile([128, (T // 128) * CL], BF16, tag=f"v{b}",
                                 name=f"v{b}")
                    for b in range(B)]

            # ---- phase 1: QKV projections + rope ----
            with tc.tile_pool(name="psqkv", bufs=1, space="PSUM") as psq:
                # dummy matmuls on ones (no DMA deps) ramp the PE p-state
                # to full clock while the first weight/x DMAs land; they
                # borrow the pv0 bank (bufs=1: same bank as the real pv0)
                warm_ps = psq.tile([128, 512], F32, tag="pv0", name="warm")
                for _ in range(24):
                    nc.tensor.matmul(warm_ps[:, 0:128], ones_sb[:],
                                     ones_sb[:], start=True, stop=True)
                for tb in range(TB):
                    tcol = tb * 512
                    rcol = tcol % T          # rope table column (per batch)
                    ps_q = [psq.tile([128, 512], F32, tag=f"pq{h}", name=f"pq{h}")
                            for h in range(HL)]
                    ps_k = [psq.tile([128, 512], F32, tag=f"pk{h}", name=f"pk{h}")
                            for h in range(HL)]
                    ps_v = [psq.tile([128, CL], F32, tag=f"pv{i}", name=f"pv{i}")
                            for i in range(4)]
                    for ct in range(CT):
                        if tb == 0:
                            xs = xs0_tiles[ct]
                        else:
                            xs = xspool.tile([128, 512], BF16, tag="xs",
                                             bufs=24)
                            eng = (nc.sync, nc.scalar, nc.gpsimd)[ct % 3]
                            eng.dma_start(
                                out=xs[:],
                                in_=xt_d[ct * 128:(ct + 1) * 128,
                                         tcol:tcol + 512],
                            )
                        st, sp = ct == 0, ct == CT - 1
                        for h in range(HL):
                            nc.tensor.matmul(
                                ps_q[h][:],
                                wq_sb[:, ct * CL + h * D: ct * CL + (h + 1) * D],
                                xs[:], start=st, stop=sp)
                            nc.tensor.matmul(
                                ps_k[h][:],
                                wk_sb[:, ct * CL + h * D: ct * CL + (h + 1) * D],
                                xs[:], start=st, stop=sp)
                        for i in range(4):
                            nc.tensor.matmul(
                                ps_v[i][:],
                                xs[:, i * 128:(i + 1) * 128],
                                wv_sb[:, ct * CL:(ct + 1) * CL],
                                start=st, stop=sp)
                    # Drain ALL psum banks first: 4 rope staging copies (h=0
                    # on DVE, h=1 on ActE — GpSimd cannot read PSUM), then v
                    # copies on ActE. Rope math afterwards reads the copies:
                    # s = tmp*ssin, shift64 via two HWDGE DMAs, dst =
                    # tmp*cos + shift64(s). h=0 math on DVE, h=1 on Pool.
                    bb = tb // 2
                    staged = []
                    for h in range(HL):
                        for ps, dst in ((ps_q[h], qT[h][bb]),
                                        (ps_k[h], kT[h][bb])):
                            tmp = ropepool.tile([128, 512], BF16, tag="rtmp",
                                                bufs=4)
                            if h == 0:
                                nc.vector.tensor_copy(tmp[:], ps[:])
                            else:
                                nc.scalar.activation(
                                    tmp[:], ps[:],
                                    mybir.ActivationFunctionType.Copy)
                            staged.append((tmp, dst))
                    for i in range(4):
                        gt = (tb % 2) * 4 + i
                        nc.scalar.activation(
                            v_sb[bb][:, gt * CL:(gt + 1) * CL], ps_v[i][:],
                            mybir.ActivationFunctionType.Copy)
                    # rope math in bf16 at 2x DVE rate
                    for tmp, dst in staged:
                        s = ropepool.tile([128, 512], BF16, tag="rs",
                                          bufs=4)
                        nc.vector.tensor_mul(
                            s[:], tmp[:], sin_sb[:, rcol:rcol + 512])
                        rot = ropepool.tile([128, 512], BF16, tag="rrot",
                                            bufs=4)
                        nc.sync.dma_start(out=rot[0:64, :],
                                          in_=s[64:128, :])
                        nc.scalar.dma_start(out=rot[64:128, :],
                                            in_=s[0:64, :])
                        u = ropepool.tile([128, 512], BF16, tag="ru",
                                          bufs=4)
                        nc.vector.tensor_mul(
                            u[:], tmp[:], cos_sb[:, rcol:rcol + 512])
                        nc.vector.tensor_add(
                            dst[:, rcol:rcol + 512], u[:], rot[:])

            # ---- phase 2: attention (block-outer, head-inner) + fused
            # partial output projection, pipelined one block behind so proj
            # never stalls on the reciprocal chain. The j-loop issues scores
            # two tiles ahead of the l/y matmuls so the in-order PE queue
            # rides out the exp-chain latency. ----
            def emit_proj(blk):
                b, jj, yb = blk
                row0 = 0 if smallout else b * T + jj * 512
                for tc_ in range(4):
                    for oq in range(4):
                        ps_o = psa.tile([128, 512], F32, tag="o", bufs=2)
                        for h in range(HL):
                            nc.tensor.matmul(
                                ps_o[:],
                                yb[h][:, tc_ * 128:(tc_ + 1) * 128],
                                wo_sb[:, h * C + oq * 512:
                                      h * C + (oq + 1) * 512],
                                start=(h == 0), stop=(h == HL - 1))
                        # ob copies alternate DVE/Act; out-DMAs on
                        # sync/gpsimd (Act queue mostly stays with exps)
                        ob = obpool.tile([128, 512], BF16, tag="ob")
                        if oq % 2 == 0:
                            nc.vector.tensor_copy(ob[:], ps_o[:])
                        else:
                            nc.scalar.activation(
                                ob[:], ps_o[:],
                                mybir.ActivationFunctionType.Copy)
                        r0 = row0 + (tc_ % 2 if smallout else tc_) * 128
                        eng = (nc.sync, nc.gpsimd)[oq % 2]
                        eng.dma_start(
                            out=out_d[r0:r0 + 128,
                                      oq * 512:(oq + 1) * 512],
                            in_=ob[:])

            with tc.tile_pool(name="psatt", bufs=1, space="PSUM") as psa:
                # keep the PE clock hot across the QKV->attention drain
                warm2 = psa.tile([128, 512], F32, tag="o", bufs=2,
                                 name="warm2")
                for _ in range(8):
                    nc.tensor.matmul(warm2[:], ones_sb[:],
                                     ones_w[:], start=True, stop=True)
                prev_blk = None
                for b, jj in ((0, 0), (1, 0), (0, 1), (1, 1)):
                    lcol = jj * 512
                    njt = 4 * jj + 4
                    yb = [None, None]
                    ps_y = [psa.tile([128, 512], F32, tag=f"y{h}", bufs=1,
                                     name=f"y{h}") for h in range(HL)]
                    ps_l = [psa.tile([128, 512], F32, tag=f"l{h}", bufs=1,
                                     name=f"l{h}") for h in range(HL)]

                    def flush(ent):
                        h, j, p, c0 = ent
                        st, sp = j == 0, j == njt - 1
                        nc.tensor.matmul(
                            ps_l[h][:, c0:512], ones_sb[:],
                            p[:, c0:512], start=st, stop=sp)
                        nc.tensor.matmul(
                            ps_y[h][:, c0:512],
                            v_sb[b][:, j * CL + h * D:
                                     j * CL + (h + 1) * D],
                            p[:, c0:512], start=st, stop=sp)

                    # heads interleaved: each head's exp latency hides
                    # behind the other head's scores + l/y matmuls
                    pend = []
                    for j in range(njt):
                        c0 = max(0, j * 128 - jj * 512)
                        diag0 = j * 128 - jj * 512
                        for h in range(HL):
                            ps_s = psa.tile([128, 512], F32, tag="s",
                                            bufs=2)
                            nc.tensor.matmul(
                                ps_s[:, c0:512],
                                kT[h][b][:, j * 128:(j + 1) * 128],
                                qT[h][b][:, lcol + c0: lcol + 512],
                                start=True, stop=True)
                            if 0 <= diag0 < 512:
                                nc.vector.tensor_add(
                                    ps_s[:, diag0:diag0 + 128],
                                    ps_s[:, diag0:diag0 + 128],
                                    mask_sb[:])
                            p = attpool.tile([128, 512], BF16, tag="p",
                                             bufs=4)
                            nc.scalar.activation(
                                p[:, c0:512], ps_s[:, c0:512],
                                mybir.ActivationFunctionType.Exp,
                                scale=ATT_SCALE)
                            pend.append((h, j, p, c0))
                            if len(pend) > 2:
                                flush(pend.pop(0))
                    for ent in pend:
                        flush(ent)
                    for h in range(HL):
                        rec = attpool.tile([128, 512], F32, tag="rec",
                                           bufs=2)
                        nc.vector.reciprocal(rec[:], ps_l[h][:])
                        # normalized y in bf16: [128 y-chans of h, 512 tok]
                        yb[h] = attpool.tile([128, 512], BF16, tag=f"yb{h}",
                                             bufs=2, name=f"yb{h}")
                        nc.vector.tensor_mul(yb[h][:], ps_y[h][:], rec[:])
                    # prev block's proj AFTER rec/yb so those get DVE queue
                    # priority; its matmuls fill PE while rec/yb drain
                    if prev_blk is not None:
                        emit_proj(prev_blk)
                    prev_blk = (b, jj, yb)
                emit_proj(prev_blk)

    nc.compile()
    return nc


_NC_CACHE = None


def _get_nc():
    global _NC_CACHE
    if _NC_CACHE is None:
        _NC_CACHE = _build()
    return _NC_CACHE


def make_in_maps(x, Wq, Wk, Wv, Wo):
    import ml_dtypes

    def conv(a):
        return np.ascontiguousarray(a).astype(ml_dtypes.bfloat16)

    x = np.asarray(x, dtype=np.float32)
    xT = conv(x.reshape(BT, C).T)                        # [C, BT] bf16

    def wlay(wT, cols):
        # [C, cols] -> [128, CT*cols]: partition p holds WT[ct*128+p, :]
        return np.ascontiguousarray(
            wT.reshape(CT, 128, cols).transpose(1, 0, 2).reshape(
                128, CT * cols))

    WoT = np.asarray(Wo, dtype=np.float32).T             # [C, C]
    in_maps = []
    for m in range(NCORES):
        sl = slice(m * CL, (m + 1) * CL)
        # my Wo^T rows [m*CL, m*CL+256) -> [128, HL*C]: partition p, col
        # block h*C+o = WoT[m*CL + h*128 + p, o]
        wo_loc = np.ascontiguousarray(
            WoT[sl, :].reshape(HL, 128, C).transpose(1, 0, 2).reshape(
                128, HL * C))
        in_maps.append({
            "xt": xT,
            "wqT": conv(wlay(np.asarray(Wq)[sl, :].T, CL)),
            "wkT": conv(wlay(np.asarray(Wk)[sl, :].T, CL)),
            "wvT": conv(wlay(np.asarray(Wv)[sl, :].T, CL)),
            "woT": conv(wo_loc),
        })
    return in_maps


def kernel(x, Wq, Wk, Wv, Wo, _trace=False):
    in_maps = make_in_maps(x, Wq, Wk, Wv, Wo)
    nc = _get_nc()
    res = run_bass_kernel_spmd(nc, in_maps, list(range(NCORES)),
                               trace=_trace)
    acc = np.zeros((BT, C), dtype=np.float32)
    for m in range(NCORES):
        acc += res.results[m]["out"].astype(np.float32)
    out = acc.reshape(B, T, C)
    if _trace:
        return out, res
    return out
